# revision 1
# baseline (speedup 1.0000x reference)
"""Gabor-atom additive audio synthesis on 8 Trainium2 NeuronCores.

Math: waveform[t] = sum_n amp_n * exp(-0.5*((t-tau_n)/sigma_n)^2)
                    * cos(2*pi*omega_n*(t-tau_n) + gamma_n*(t-tau_n)^2 + phi_n)
with N=2048 atoms, T=48000 samples (2s @ 24kHz).

Sharding: atoms across 8 cores (256 atoms/core = 2 blocks of 128 partitions).
Per core, time is tiled (F=768). For each (block, tile) the per-element work is:
  - envelope log-arg q and phase-in-cycles y are evaluated as quadratics in
    local-time u via PE matmuls whose stationary rows are bf16 multi-split
    coefficients (guaranteed ~2^-25 relative precision, full PE rate)
  - env' = amp*exp(q) via one ACT Exp (amp folded in as ln(amp))
  - y -> frac = y - round(y) via DVE int32 round + mixed-dtype subtract
  - cos = Sin(2*pi*frac) via one ACT Sin (the +0.25 cycle shift is folded
    into the phase constant so Sin directly yields the cosine)
  - m = env'*cos on GPSIMD; PE reduce (stationary=m chunk, moving=ones)
    accumulates waveform columns into a single PSUM bank [128, 375]
ACT work is batched by table set (Exp... then Sin...) to avoid table thrash.
Host: fp64 coefficient prep, final 8-way partial sum.
"""
import numpy as np
import ml_dtypes
from contextlib import ExitStack

import concourse.bacc as bacc
import concourse.tile as tile
from concourse import mybir
from concourse.bass_utils import run_bass_kernel_spmd
from concourse.tile import add_dep_helper

# ---- problem constants (hardcoded; kernel.py must be self-contained) ----
FS = 24000.0
T = 48000
N_ATOMS = 2048
N_CORES = 8
NYQUIST = FS / 2.0
SIGMA_OFFSET = 1e-3

P = 128                      # partitions / atoms per block
BLOCKS = 2                   # atom blocks per core (256 atoms/core)
F = 768                      # time-tile width (u_max = 767/24000 ~ 0.032 s)
N_TILES = T // F + (1 if T % F else 0)      # 63 (62 full + 384 remainder)
REM = T - (N_TILES - 1) * F                 # 384
CHUNK = 128                  # reduce chunk (output column) width
N_COLS = T // CHUNK          # 375 output columns
KQ = 9                       # envelope matmul contraction rows
KP = 8                       # phase matmul contraction rows
BATCH = 5
ENV_BUFS = 14
FRAC_BUFS = 14
MM_N = 384                   # matmul free-dim chunk (<=512, PSUM bank limit)

f32 = mybir.dt.float32
i32 = mybir.dt.int32
bf16 = mybir.dt.bfloat16
bft = ml_dtypes.bfloat16

_cache = {}


def _bsplit(x, n):
    """Split fp64 array into n bf16 terms summing to ~2^-(9n) rel precision."""
    parts, r = [], np.asarray(x, np.float64).copy()
    for _ in range(n):
        p = r.astype(bft)
        parts.append(p)
        r = r - p.astype(np.float64)
    return parts


def _build_program():
    nc = bacc.Bacc("TRN2", target_bir_lowering=False, debug=False)

    d_movq = nc.dram_tensor("mov_q", [KQ, F], bf16, kind="ExternalInput").ap()
    d_movp = nc.dram_tensor("mov_p", [KP, F], bf16, kind="ExternalInput").ap()
    d_statq = nc.dram_tensor("stat_q", [N_TILES, KQ, BLOCKS * P], bf16,
                             kind="ExternalInput").ap()
    d_statp = nc.dram_tensor("stat_p", [N_TILES, KP, BLOCKS * P], bf16,
                             kind="ExternalInput").ap()
    d_out = nc.dram_tensor("wave", [P, N_COLS], f32, kind="ExternalOutput").ap()

    act_chain = []  # explicit ACT-stream order: batches of Exp, then Sin
    with tile.TileContext(nc) as tc, ExitStack() as ctx:
        consts = ctx.enter_context(tc.tile_pool(name="consts", bufs=1))
        statpool = ctx.enter_context(tc.tile_pool(name="stats", bufs=4))
        envpool = ctx.enter_context(tc.tile_pool(name="env", bufs=ENV_BUFS))
        fracpool = ctx.enter_context(tc.tile_pool(name="frac", bufs=FRAC_BUFS))
        kpool = ctx.enter_context(tc.tile_pool(name="kint", bufs=5))
        sinpool = ctx.enter_context(tc.tile_pool(name="sin", bufs=4))
        mpool = ctx.enter_context(tc.tile_pool(name="m", bufs=4))
        opool = ctx.enter_context(tc.tile_pool(name="ocopy", bufs=1))
        qppool = ctx.enter_context(tc.tile_pool(name="qp", bufs=2, space="PSUM"))
        outpool = ctx.enter_context(tc.tile_pool(name="outp", bufs=1, space="PSUM"))

        t_movq = consts.tile([KQ, F], bf16)
        nc.sync.dma_start(t_movq[:], d_movq[:])
        t_movp = consts.tile([KP, F], bf16)
        nc.gpsimd.dma_start(t_movp[:], d_movp[:])
        t_ones = consts.tile([P, 1], bf16)
        nc.vector.memset(t_ones[:], 1.0)

        p_out = outpool.tile([P, 512], f32)

        def tf(i):  # tile i free width
            return REM if i == N_TILES - 1 else F

        # ---- per-tile stages ----
        def stage_a(i):
            """matmuls -> env (ACT Exp) + frac (DVE). Returns (env, frac) tiles."""
            w = tf(i)
            t_sq = statpool.tile([KQ, BLOCKS * P], bf16, tag="sq")
            nc.sync.dma_start(t_sq[:], d_statq[i])
            t_sp = statpool.tile([KP, BLOCKS * P], bf16, tag="sp")
            nc.sync.dma_start(t_sp[:], d_statp[i])

            p_q = qppool.tile([P, BLOCKS * F], f32, tag="ps")
            p_p = qppool.tile([P, BLOCKS * F], f32, tag="ps")
            # matmul PSUM writes must not straddle a 512-col bank boundary
            for b in range(BLOCKS):
                o = 0
                while o < w:
                    col = b * w + o
                    n = min(w - o, 512 - (col % 512))
                    nc.tensor.matmul(
                        p_q[:, col: col + n],
                        t_sq[:, b * P:(b + 1) * P], t_movq[:, o:o + n],
                        start=True, stop=True)
                    nc.tensor.matmul(
                        p_p[:, col: col + n],
                        t_sp[:, b * P:(b + 1) * P], t_movp[:, o:o + n],
                        start=True, stop=True)
                    o += n

            t_env = envpool.tile([P, BLOCKS * F], bf16, tag="env")
            act_chain.append(nc.scalar.activation(
                t_env[:, :BLOCKS * w], p_q[:, :BLOCKS * w],
                mybir.ActivationFunctionType.Exp))
            t_k = kpool.tile([P, BLOCKS * F], i32, tag="k")
            t_frac = fracpool.tile([P, BLOCKS * F], f32, tag="frac")
            # DVE drains PSUM and converts at 2x from SBUF; the otherwise-idle
            # GPSIMD does the round-subtract in place (engine load balancing)
            nc.vector.tensor_copy(t_frac[:, :BLOCKS * w], p_p[:, :BLOCKS * w])
            nc.vector.tensor_copy(t_k[:, :BLOCKS * w], t_frac[:, :BLOCKS * w])
            nc.gpsimd.tensor_tensor(t_frac[:, :BLOCKS * w],
                                    t_frac[:, :BLOCKS * w],
                                    t_k[:, :BLOCKS * w],
                                    mybir.AluOpType.subtract)
            return t_env, t_frac

        def stage_b(i, t_env, t_frac):
            """Sin -> mult -> reduce-accumulate into p_out columns."""
            w = tf(i)
            t_sin = sinpool.tile([P, BLOCKS * F], bf16, tag="sin")
            act_chain.append(nc.scalar.activation(
                t_sin[:, :BLOCKS * w], t_frac[:, :BLOCKS * w],
                mybir.ActivationFunctionType.Sin, scale=2.0 * np.pi))
            t_m = mpool.tile([P, BLOCKS * F], bf16, tag="m")
            nc.vector.tensor_tensor(t_m[:, :BLOCKS * w], t_env[:, :BLOCKS * w],
                                    t_sin[:, :BLOCKS * w], mybir.AluOpType.mult)
            ncols = w // CHUNK
            # keep the accumulating pair adjacent: a start=True matmul resets
            # accumulation state bank-wide, so groups must not interleave
            for j in range(ncols):
                for b in range(BLOCKS):
                    c = (i * F) // CHUNK + j
                    nc.tensor.matmul(
                        p_out[:, c:c + 1],
                        t_m[:, b * w + j * CHUNK: b * w + (j + 1) * CHUNK],
                        t_ones[:],
                        start=(b == 0), stop=(b == BLOCKS - 1))

        prev = []
        i = 0
        while i < N_TILES:
            hi = min(i + BATCH, N_TILES)
            cur = [(j, *stage_a(j)) for j in range(i, hi)]
            for j, t_env, t_frac in prev:
                stage_b(j, t_env, t_frac)
            prev = cur
            i = hi
        for j, t_env, t_frac in prev:
            stage_b(j, t_env, t_frac)

        t_w = opool.tile([P, N_COLS], f32)
        act_chain.append(nc.scalar.copy(t_w[:], p_out[:, :N_COLS]))
        nc.sync.dma_start(d_out[:], t_w[:])
        for prev, nxt in zip(act_chain[:-1], act_chain[1:]):
            add_dep_helper(nxt.ins, prev.ins, sync=False,
                           reason="ACT table-set batching order")

    nc.compile()
    return nc


def _prepare_inputs(amplitude_logit, tau, omega_logit, sigma_logit,
                    phi_vector, gamma):
    """fp64 host prep -> per-core input maps."""
    al = amplitude_logit.astype(np.float64)
    tau = tau.astype(np.float64)
    ol = omega_logit.astype(np.float64)
    sl = sigma_logit.astype(np.float64)
    pv = phi_vector.astype(np.float64)
    gamma = gamma.astype(np.float64)

    ln_amp = np.where(al > 30, al, np.log(np.log1p(np.exp(al))))
    omega = (1.0 / (1.0 + np.exp(-ol))) * 0.99 * NYQUIST
    sigma = np.where(sl > 30, sl, np.log1p(np.exp(sl))) + SIGMA_OFFSET
    phi = np.arctan2(pv[:, 1], pv[:, 0])

    # shared moving rows (local time u = j/FS, exact grid)
    j = np.arange(F, dtype=np.float64)
    u = j / FS
    w2 = u * u
    u1, u2, u3 = _bsplit(u, 3)
    w1, w2b = _bsplit(w2, 2)
    one = np.ones(F, dtype=bft)
    mov_q = np.stack([one, one, u1, u1, u2, u2, w1, w1, w2b])
    mov_p = np.stack([one, one, u1, u1, u1, u2, u2, u3])

    t0s = (np.arange(N_TILES, dtype=np.float64) * F) / FS          # [I]
    in_maps = []
    for c in range(N_CORES):
        sel = slice(c * (N_ATOMS // N_CORES), (c + 1) * (N_ATOMS // N_CORES))
        tau_c, sig_c, om_c = tau[sel], sigma[sel], omega[sel]
        ga_c, phi_c, la_c = gamma[sel], phi[sel], ln_amp[sel]

        D = t0s[:, None] - tau_c[None, :]                           # [I, 256]
        inv_s2 = 1.0 / (sig_c * sig_c)
        c0 = -0.5 * D * D * inv_s2[None, :] + la_c[None, :]
        c1 = -D * inv_s2[None, :]
        c2 = np.broadcast_to(-0.5 * inv_s2[None, :], D.shape)
        Bc = om_c[None, :] + ga_c[None, :] * D / np.pi
        C = (om_c[None, :] * D + ga_c[None, :] * D * D / (2 * np.pi)
             + phi_c[None, :] / (2 * np.pi) + 0.25)
        C = C - np.round(C)

        c0_1, c0_2 = _bsplit(c0, 2)
        c1_1, c1_2 = _bsplit(c1, 2)
        c2_1, c2_2 = _bsplit(c2, 2)
        C1, C2 = _bsplit(C, 2)
        B1, B2, B3 = _bsplit(Bc, 3)

        stat_q = np.stack([c0_1, c0_2, c1_1, c1_2, c1_1, c1_2,
                           c2_1, c2_2, c2_1], axis=1)               # [I, 9, 256]
        stat_p = np.stack([C1, C2, B1, B2, B3, B1, B2, B1], axis=1)  # [I, 8, 256]
        in_maps.append({
            "mov_q": np.ascontiguousarray(mov_q),
            "mov_p": np.ascontiguousarray(mov_p),
            "stat_q": np.ascontiguousarray(stat_q),
            "stat_p": np.ascontiguousarray(stat_p),
        })
    return in_maps


def kernel(amplitude_logit, tau, omega_logit, sigma_logit, phi_vector, gamma, t):
    if "nc" not in _cache:
        _cache["nc"] = _build_program()
    nc = _cache["nc"]
    in_maps = _prepare_inputs(amplitude_logit, tau, omega_logit, sigma_logit,
                              phi_vector, gamma)
    res = run_bass_kernel_spmd(nc, in_maps, list(range(N_CORES)))
    total = np.zeros(T, dtype=np.float64)
    for r in res.results:
        wv = r["wave"].astype(np.float64)          # [P, N_COLS]
        total += wv.T.ravel()                      # sample s = c*128 + p
    return total.astype(np.float32)



# revision 2
# speedup vs baseline: 2.6237x; 2.6237x over previous
"""Gabor-atom additive audio synthesis on 8 Trainium2 NeuronCores.

Math: waveform[t] = sum_n amp_n * exp(-0.5*((t-tau_n)/sigma_n)^2)
                    * cos(2*pi*omega_n*(t-tau_n) + gamma_n*(t-tau_n)^2 + phi_n)
with N=2048 atoms, T=48000 samples (2s @ 24kHz).

Sharding: atoms across 8 cores (256 atoms/core = 2 blocks of 128 partitions).

Per-core pipeline (DDS-style fixed-point phase, 1 cycle = 2^23):
  - DVE/GPSIMD tensor_scalar FMA: k = round_i32(Bi[p]*i' + Ci[p]) per
    960-sample chunk (centered ramp i' in [-480,480) keeps |v| < 2^31; Bi/Ci
    are integer-valued f32 so the f32 FMA is exact to ~2^-24 of the phase)
  - DVE fused (k & 0x7FFFFF) | 0x3F800000: IEEE bits of 1 + frac(phase);
    bitcast to f32 gives the fractional phase linearly in [1,2)
  - ACT Sin(2*pi*x - 3*pi) on the bitcast: -sin(2*pi*frac) = -cos(orig phase)
    (the +0.25-cycle cos->sin shift is folded into Ci on the host)
  - PE reduce per 128-sample chunk: out[i,{0,1}] = sum_p sin[p,i]*envp[p,{0,1}]
    with envp = (-amp*env(node c), -amp*(env(node c+1)-env(node c))) host-
    computed at chunk boundaries (the Gaussian env is smooth at 5.3ms scale)
  - DVE combine: wave[i,c] = r0 + (i/128)*rd  (linear env interpolation)
Host: fp64 transforms + per-chunk phase/envelope constants, final 8-way sum.
"""
import numpy as np
import ml_dtypes
from contextlib import ExitStack

import concourse.bacc as bacc
import concourse.tile as tile
from concourse import mybir
from concourse.bass_utils import run_bass_kernel_spmd

# ---- problem constants (hardcoded; kernel.py must be self-contained) ----
FS = 24000.0
T = 48000
N_ATOMS = 2048
N_CORES = 8
NYQUIST = FS / 2.0
SIGMA_OFFSET = 1e-3

P = 128                      # partitions / atoms per block
BLOCKS = 2                   # atom blocks per core (256 atoms/core)
FIX = 2 ** 23                # fixed-point: 1 cycle = 2^23
CH = 960                     # FMA chunk (|Bi*i'| <= .495*2^23*480 < 2^31)
N_CH = T // CH               # 100 FMA chunks
F = 1920                     # processing tile = 2 FMA chunks
N_TILES = T // F             # 25
RCH = 128                    # reduce chunk (output column height)
N_COLS = T // RCH            # 375
RPT = F // RCH               # reduce chunks per tile (15)
GPS_NUM, GPS_DEN = 11, 20    # GPSIMD takes 11/20 of FMA items

f32 = mybir.dt.float32
i32 = mybir.dt.int32
bf16 = mybir.dt.bfloat16
bft = ml_dtypes.bfloat16

_cache = {}


def _build_program():
    nc = bacc.Bacc("TRN2", target_bir_lowering=False, debug=False)

    d_bi = nc.dram_tensor("bi", [P, BLOCKS * N_CH], f32, kind="ExternalInput").ap()
    d_ci = nc.dram_tensor("ci", [P, BLOCKS * N_CH], f32, kind="ExternalInput").ap()
    d_envp = nc.dram_tensor("envp", [P, BLOCKS * 2 * N_COLS], bf16,
                            kind="ExternalInput").ap()
    d_out = nc.dram_tensor("wave", [P, N_COLS], f32, kind="ExternalOutput").ap()

    with tile.TileContext(nc) as tc, ExitStack() as ctx:
        consts = ctx.enter_context(tc.tile_pool(name="consts", bufs=1))
        kpool = ctx.enter_context(tc.tile_pool(name="kp", bufs=3))
        mpool = ctx.enter_context(tc.tile_pool(name="mp", bufs=3))
        spool = ctx.enter_context(tc.tile_pool(name="sp", bufs=3))
        opool = ctx.enter_context(tc.tile_pool(name="op", bufs=1))
        rpool = ctx.enter_context(tc.tile_pool(name="rp", bufs=1, space="PSUM"))

        t_bi = consts.tile([P, BLOCKS * N_CH], f32)
        nc.gpsimd.dma_start(t_bi[:], d_bi[:])
        t_ci = consts.tile([P, BLOCKS * N_CH], f32)
        nc.gpsimd.dma_start(t_ci[:], d_ci[:])
        # env pairs in two DMAs so early reduce chunks aren't gated on the tail
        t_envp = consts.tile([P, BLOCKS * 2 * N_COLS], bf16)
        half = N_COLS  # half the pairs of each block
        nc.sync.dma_start(t_envp[:, :half], d_envp[:, :half])
        nc.sync.dma_start(t_envp[:, BLOCKS * N_COLS:BLOCKS * N_COLS + half],
                          d_envp[:, BLOCKS * N_COLS:BLOCKS * N_COLS + half])
        nc.sync.dma_start(t_envp[:, half:BLOCKS * N_COLS],
                          d_envp[:, half:BLOCKS * N_COLS])
        nc.sync.dma_start(t_envp[:, BLOCKS * N_COLS + half:],
                          d_envp[:, BLOCKS * N_COLS + half:])

        # centered local ramp i' in [-CH/2, CH/2), as exact-integer f32
        t_ji = consts.tile([P, CH], i32)
        nc.gpsimd.iota(t_ji[:], [[1, CH]], base=-CH // 2, channel_multiplier=0)
        t_j = consts.tile([P, CH], f32)
        nc.vector.tensor_copy(t_j[:], t_ji[:])
        # per-partition ramp weight i/128 for the env interpolation
        t_pi = consts.tile([P, 1], i32)
        nc.gpsimd.iota(t_pi[:], [[0, 1]], base=0, channel_multiplier=1)
        t_w = consts.tile([P, 1], f32)
        nc.vector.tensor_scalar(t_w[:], t_pi[:], float(1.0 / RCH), None,
                                mybir.AluOpType.mult)
        t_bias = consts.tile([P, 1], f32)
        nc.vector.memset(t_bias[:], float(-3.0 * np.pi))

        p_r = rpool.tile([P, 2 * N_COLS], f32)

        q = 0  # global FMA item index for DVE/GPSIMD split
        for t in range(N_TILES):
            t_k = kpool.tile([P, BLOCKS * F], i32, tag="k")
            for b in range(BLOCKS):
                for h in range(2):
                    c960 = t * 2 + h
                    col = b * N_CH + c960
                    eng = nc.gpsimd if (q * GPS_NUM) % GPS_DEN < GPS_NUM \
                        else nc.vector
                    eng.tensor_scalar(
                        t_k[:, b * F + h * CH: b * F + (h + 1) * CH],
                        t_j[:], t_bi[:, col:col + 1], t_ci[:, col:col + 1],
                        mybir.AluOpType.mult, mybir.AluOpType.add)
                    q += 1
            t_m = mpool.tile([P, BLOCKS * F], i32, tag="m")
            nc.vector.tensor_scalar(t_m[:], t_k[:], 0x7FFFFF, 0x3F800000,
                                    mybir.AluOpType.bitwise_and,
                                    mybir.AluOpType.bitwise_or)
            t_s = spool.tile([P, BLOCKS * F], bf16, tag="s")
            nc.scalar.activation(t_s[:], t_m[:].bitcast(f32),
                                 mybir.ActivationFunctionType.Sin,
                                 scale=float(2.0 * np.pi),
                                 bias=t_bias[:, 0:1])
            for r in range(RPT):
                c = t * RPT + r
                for b in range(BLOCKS):
                    nc.tensor.matmul(
                        p_r[:, 2 * c:2 * c + 2],
                        t_s[:, b * F + r * RCH: b * F + (r + 1) * RCH],
                        t_envp[:, b * 2 * N_COLS + 2 * c:
                               b * 2 * N_COLS + 2 * c + 2],
                        start=(b == 0), stop=(b == BLOCKS - 1))

        # wave[i,c] = r0[i,c] + (i/128)*rd[i,c]
        t_tmp = opool.tile([P, N_COLS], f32)
        nc.vector.tensor_scalar(
            t_tmp[:], p_r[:, 1:2 * N_COLS:2], t_w[:, 0:1], None,
            mybir.AluOpType.mult)
        t_wav = opool.tile([P, N_COLS], f32)
        nc.vector.tensor_tensor(t_wav[:], t_tmp[:], p_r[:, 0:2 * N_COLS:2],
                                mybir.AluOpType.add)
        nc.sync.dma_start(d_out[:], t_wav[:])

    nc.compile()
    return nc


def _prepare_inputs(amplitude_logit, tau, omega_logit, sigma_logit,
                    phi_vector, gamma):
    """fp64 host prep -> per-core input maps."""
    al = amplitude_logit.astype(np.float64)
    tau = tau.astype(np.float64)
    ol = omega_logit.astype(np.float64)
    sl = sigma_logit.astype(np.float64)
    pv = phi_vector.astype(np.float64)
    gamma = gamma.astype(np.float64)

    amp = np.where(al > 30, al, np.log1p(np.exp(al)))
    omega = (1.0 / (1.0 + np.exp(-ol))) * 0.99 * NYQUIST
    sigma = np.where(sl > 30, sl, np.log1p(np.exp(sl))) + SIGMA_OFFSET
    phi = np.arctan2(pv[:, 1], pv[:, 0])

    t_cen = (np.arange(N_CH) * CH + CH // 2) / FS            # [100]
    t_node = np.arange(N_COLS + 1) * RCH / FS                # [376]

    per_core = N_ATOMS // N_CORES
    in_maps = []
    for c in range(N_CORES):
        sel = slice(c * per_core, (c + 1) * per_core)
        tau_c, sig_c, om_c = tau[sel], sigma[sel], omega[sel]
        ga_c, phi_c, amp_c = gamma[sel], phi[sel], amp[sel]

        # phase constants per (atom, 960-chunk); y in cycles incl. +0.25
        D = t_cen[None, :] - tau_c[:, None]                  # [256, 100]
        y = om_c[:, None] * D + ga_c[:, None] * D * D / (2 * np.pi) \
            + (phi_c[:, None] / (2 * np.pi)) + 0.25
        dy = om_c[:, None] + ga_c[:, None] * D / np.pi       # cycles/sec
        bi = np.round(dy / FS * FIX)                         # fix units/sample
        ci = np.round(np.mod(y, 1.0) * FIX)                  # [0, 2^23]

        # envelope (with amp and the -sin sign fold) at chunk nodes
        E = amp_c[:, None] * np.exp(
            -0.5 * ((t_node[None, :] - tau_c[:, None]) / sig_c[:, None]) ** 2)
        e0 = -E[:, :-1]                                      # [256, 375]
        ed = -(E[:, 1:] - E[:, :-1])
        envp = np.empty((per_core, 2 * N_COLS), np.float64)
        envp[:, 0::2] = e0
        envp[:, 1::2] = ed

        def blk(x):  # [256, W] -> [128, 2*W] block-major
            w = x.shape[1]
            out = np.empty((P, BLOCKS * w), x.dtype)
            for b in range(BLOCKS):
                out[:, b * w:(b + 1) * w] = x[b * P:(b + 1) * P]
            return out

        in_maps.append({
            "bi": np.ascontiguousarray(blk(bi).astype(np.float32)),
            "ci": np.ascontiguousarray(blk(ci).astype(np.float32)),
            "envp": np.ascontiguousarray(blk(envp).astype(bft)),
        })
    return in_maps


def kernel(amplitude_logit, tau, omega_logit, sigma_logit, phi_vector, gamma, t):
    if "nc" not in _cache:
        _cache["nc"] = _build_program()
    nc = _cache["nc"]
    in_maps = _prepare_inputs(amplitude_logit, tau, omega_logit, sigma_logit,
                              phi_vector, gamma)
    res = run_bass_kernel_spmd(nc, in_maps, list(range(N_CORES)))
    total = np.zeros(T, dtype=np.float64)
    for r in res.results:
        wv = r["wave"].astype(np.float64)          # [P, N_COLS]
        total += wv.T.ravel()                      # sample s = c*128 + i
    return total.astype(np.float32)


# revision 4
# speedup vs baseline: 2.6534x; 1.0113x over previous
"""Gabor-atom additive audio synthesis on 8 Trainium2 NeuronCores.

Math: waveform[t] = sum_n amp_n * exp(-0.5*((t-tau_n)/sigma_n)^2)
                    * cos(2*pi*omega_n*(t-tau_n) + gamma_n*(t-tau_n)^2 + phi_n)
with N=2048 atoms, T=48000 samples (2s @ 24kHz).

Sharding: atoms across 8 cores (256 atoms/core = 2 blocks of 128 partitions).

Per-core pipeline (DDS-style fixed-point phase, 1 cycle = 2^23):
  - DVE/GPSIMD tensor_scalar FMA: k = round_i32(Bi[p]*i' + Ci[p]) per
    960-sample chunk (centered ramp i' in [-480,480) keeps |v| < 2^31; Bi/Ci
    are integer-valued f32 so the f32 FMA is exact to ~2^-24 of the phase)
  - DVE fused (k & 0x7FFFFF) | 0x3F800000: IEEE bits of 1 + frac(phase);
    bitcast to f32 gives the fractional phase linearly in [1,2)
  - ACT Sin(2*pi*x - 3*pi) on the bitcast: -sin(2*pi*frac) = -cos(orig phase)
    (the +0.25-cycle cos->sin shift is folded into Ci on the host)
  - PE reduce per 128-sample chunk: out[i,{0,1}] = sum_p sin[p,i]*envp[p,{0,1}]
    with envp = (-amp*env(node c), -amp*(env(node c+1)-env(node c))) host-
    computed at chunk boundaries (the Gaussian env is smooth at 5.3ms scale)
  - DVE combine: wave[i,c] = r0 + (i/128)*rd  (linear env interpolation)
Schedule: Sin table preloaded by a 1-element warm-up; tile 0 runs split in
two halves (896/1024, all-DVE FMAs) so ACT starts ~4us in; tiles 1..24 batch
Sin per 2-tile pair; the combine+output DMA runs in two halves so only half
remains after the last Sin. Host: fp64 transforms + per-chunk phase/envelope
constants, final 8-way sum.
"""
import numpy as np
import ml_dtypes
from contextlib import ExitStack

import concourse.bacc as bacc
import concourse.tile as tile
from concourse import mybir
from concourse.bass_utils import run_bass_kernel_spmd

# ---- problem constants (hardcoded; kernel.py must be self-contained) ----
FS = 24000.0
T = 48000
N_ATOMS = 2048
N_CORES = 8
NYQUIST = FS / 2.0
SIGMA_OFFSET = 1e-3

P = 128                      # partitions / atoms per block
BLOCKS = 2                   # atom blocks per core (256 atoms/core)
FIX = 2 ** 23                # fixed-point: 1 cycle = 2^23
CH = 960                     # FMA chunk (|Bi*i'| <= .495*2^23*480 < 2^31)
N_CH = T // CH               # 100 FMA chunks
F = 1920                     # processing tile = 2 FMA chunks
N_TILES = T // F             # 25
RCH = 128                    # reduce chunk (output column height)
N_COLS = T // RCH            # 375
RPT = F // RCH               # reduce chunks per tile (15)
HA = 896                     # tile-0 half A samples (7 reduce chunks)
HB = F - HA                  # tile-0 half B samples (1024, 8 chunks)

f32 = mybir.dt.float32
i32 = mybir.dt.int32
bf16 = mybir.dt.bfloat16
bft = ml_dtypes.bfloat16

_cache = {}


def _build_program():
    nc = bacc.Bacc("TRN2", target_bir_lowering=False, debug=False)

    d_bi = nc.dram_tensor("bi", [P, BLOCKS * N_CH], f32, kind="ExternalInput").ap()
    d_ci = nc.dram_tensor("ci", [P, BLOCKS * N_CH], f32, kind="ExternalInput").ap()
    d_envp = nc.dram_tensor("envp", [P, BLOCKS * 2 * N_COLS], bf16,
                            kind="ExternalInput").ap()
    d_out = nc.dram_tensor("wave", [P, N_COLS], f32, kind="ExternalOutput").ap()

    with tile.TileContext(nc) as tc, ExitStack() as ctx:
        consts = ctx.enter_context(tc.tile_pool(name="consts", bufs=1))
        kpool = ctx.enter_context(tc.tile_pool(name="kp", bufs=3))
        mpool = ctx.enter_context(tc.tile_pool(name="mp", bufs=2))
        spool = ctx.enter_context(tc.tile_pool(name="sp", bufs=2))
        opool = ctx.enter_context(tc.tile_pool(name="op", bufs=1))
        rpool = ctx.enter_context(tc.tile_pool(name="rp", bufs=1, space="PSUM"))

        # centered local ramp i' in [-CH/2, CH/2) first: it gates the FMAs
        # (used directly as i32 FMA input; the ALU upcasts exactly)
        t_ji = consts.tile([P, CH], i32)
        nc.gpsimd.iota(t_ji[:], [[1, CH]], base=-CH // 2, channel_multiplier=0)
        t_pi = consts.tile([P, 1], i32)
        nc.gpsimd.iota(t_pi[:], [[0, 1]], base=0, channel_multiplier=1)

        t_bias = consts.tile([P, 1], f32)
        nc.vector.memset(t_bias[:], float(-3.0 * np.pi))
        # preload the Sin act table while the pipeline fills
        t_warm = consts.tile([P, 1], bf16)
        nc.scalar.activation(t_warm[:], t_bias[:],
                             mybir.ActivationFunctionType.Sin, scale=1.0)

        t_bi = consts.tile([P, BLOCKS * N_CH], f32)
        nc.sync.dma_start(t_bi[:], d_bi[:])
        t_ci = consts.tile([P, BLOCKS * N_CH], f32)
        nc.sync.dma_start(t_ci[:], d_ci[:])
        t_envp = consts.tile([P, BLOCKS * 2 * N_COLS], bf16)
        half = N_COLS  # half the pairs of each block
        nc.sync.dma_start(t_envp[:, :half], d_envp[:, :half])
        nc.sync.dma_start(t_envp[:, BLOCKS * N_COLS:BLOCKS * N_COLS + half],
                          d_envp[:, BLOCKS * N_COLS:BLOCKS * N_COLS + half])
        nc.sync.dma_start(t_envp[:, half:BLOCKS * N_COLS],
                          d_envp[:, half:BLOCKS * N_COLS])
        nc.sync.dma_start(t_envp[:, BLOCKS * N_COLS + half:],
                          d_envp[:, BLOCKS * N_COLS + half:])

        # per-partition ramp weight i/128 for the env interpolation
        t_w = consts.tile([P, 1], f32)
        nc.vector.tensor_scalar(t_w[:], t_pi[:], float(1.0 / RCH), None,
                                mybir.AluOpType.mult)

        p_r = rpool.tile([P, 2 * N_COLS], f32)
        t_wav = opool.tile([P, N_COLS], f32)

        # GPSIMD takes 54 of the 96 FMA items of tiles 1..24 (9/16);
        # tile 0's 4 items stay on DVE for the fastest possible head.
        def pick_engine(q):
            return nc.gpsimd if (q * 9) % 16 < 9 else nc.vector

        def fma_item(t_k, t, b, h, eng):
            col = b * N_CH + t * 2 + h
            eng.tensor_scalar(
                t_k[:, b * F + h * CH: b * F + (h + 1) * CH],
                t_ji[:], t_bi[:, col:col + 1], t_ci[:, col:col + 1],
                mybir.AluOpType.mult, mybir.AluOpType.add)

        def andor(t_m, moff, t_k, koff, w):
            nc.vector.tensor_scalar(
                t_m[:, moff:moff + w], t_k[:, koff:koff + w],
                0x7FFFFF, 0x3F800000,
                mybir.AluOpType.bitwise_and, mybir.AluOpType.bitwise_or)

        def sin(t_s, t_m, off, w):
            nc.scalar.activation(
                t_s[:, off:off + w], t_m[:, off:off + w].bitcast(f32),
                mybir.ActivationFunctionType.Sin,
                scale=float(2.0 * np.pi), bias=t_bias[:, 0:1])

        def rmm(c, stat_ap, b):
            nc.tensor.matmul(
                p_r[:, 2 * c:2 * c + 2], stat_ap,
                t_envp[:, b * 2 * N_COLS + 2 * c: b * 2 * N_COLS + 2 * c + 2],
                start=(b == 0), stop=(b == BLOCKS - 1))

        def combine(lo, hi):  # wave[i,c] = r0[i,c] + (i/128)*rd[i,c]
            t_tmp = opool.tile([P, N_COLS], f32, tag="tmp")
            nc.vector.tensor_scalar(
                t_tmp[:, lo:hi], p_r[:, 2 * lo + 1:2 * hi:2], t_w[:, 0:1],
                None, mybir.AluOpType.mult)
            nc.vector.tensor_tensor(t_wav[:, lo:hi], t_tmp[:, lo:hi],
                                    p_r[:, 2 * lo:2 * hi:2],
                                    mybir.AluOpType.add)
            nc.sync.dma_start(d_out[:, lo:hi], t_wav[:, lo:hi])

        # --- tile 0: split halves (A: samples 0..895, B: 896..1919) ---
        t_k0 = kpool.tile([P, BLOCKS * F], i32, tag="k")
        for b in range(BLOCKS):          # h=0 chunks first: half A dep
            fma_item(t_k0, 0, b, 0, nc.vector)
        for b in range(BLOCKS):
            fma_item(t_k0, 0, b, 1, nc.vector)
        t_m0 = mpool.tile([P, 2 * BLOCKS * F], i32, tag="m")
        t_s0 = spool.tile([P, 2 * BLOCKS * F], bf16, tag="s")
        for b in range(BLOCKS):          # half A: needs only h=0 FMAs
            andor(t_m0, b * HA, t_k0, b * F, HA)
        sin(t_s0, t_m0, 0, BLOCKS * HA)
        for b in range(BLOCKS):          # half B
            andor(t_m0, BLOCKS * HA + b * HB, t_k0, b * F + HA, HB)
        sin(t_s0, t_m0, BLOCKS * HA, BLOCKS * HB)
        for r in range(RPT):
            for b in range(BLOCKS):
                if r < HA // RCH:
                    off = b * HA + r * RCH
                else:
                    off = BLOCKS * HA + b * HB + (r - HA // RCH) * RCH
                rmm(r, t_s0[:, off:off + RCH], b)

        # --- tiles 1..24: Sin batched per pair ---
        q = 0
        t = 1
        while t < N_TILES:
            pair = min(2, N_TILES - t)
            t_m = mpool.tile([P, 2 * BLOCKS * F], i32, tag="m")
            t_s = spool.tile([P, 2 * BLOCKS * F], bf16, tag="s")
            for pi in range(pair):
                t_k = kpool.tile([P, BLOCKS * F], i32, tag="k")
                for b in range(BLOCKS):
                    for h in range(2):
                        fma_item(t_k, t + pi, b, h, pick_engine(q))
                        q += 1
                andor(t_m, pi * BLOCKS * F, t_k, 0, BLOCKS * F)
            sin(t_s, t_m, 0, pair * BLOCKS * F)
            for pi in range(pair):
                for r in range(RPT):
                    c = (t + pi) * RPT + r
                    b_off = pi * BLOCKS * F
                    for b in range(BLOCKS):
                        rmm(c, t_s[:, b_off + b * F + r * RCH:
                                   b_off + b * F + (r + 1) * RCH], b)
            t += pair
            if t == 13:                  # chunks 0..194 are final
                combine(0, 13 * RPT)

        combine(13 * RPT, N_COLS)

    nc.compile()
    return nc


def _prepare_inputs(amplitude_logit, tau, omega_logit, sigma_logit,
                    phi_vector, gamma):
    """fp64 host prep -> per-core input maps."""
    al = amplitude_logit.astype(np.float64)
    tau = tau.astype(np.float64)
    ol = omega_logit.astype(np.float64)
    sl = sigma_logit.astype(np.float64)
    pv = phi_vector.astype(np.float64)
    gamma = gamma.astype(np.float64)

    amp = np.where(al > 30, al, np.log1p(np.exp(al)))
    omega = (1.0 / (1.0 + np.exp(-ol))) * 0.99 * NYQUIST
    sigma = np.where(sl > 30, sl, np.log1p(np.exp(sl))) + SIGMA_OFFSET
    phi = np.arctan2(pv[:, 1], pv[:, 0])

    t_cen = (np.arange(N_CH) * CH + CH // 2) / FS            # [100]
    t_node = np.arange(N_COLS + 1) * RCH / FS                # [376]

    per_core = N_ATOMS // N_CORES
    in_maps = []
    for c in range(N_CORES):
        sel = slice(c * per_core, (c + 1) * per_core)
        tau_c, sig_c, om_c = tau[sel], sigma[sel], omega[sel]
        ga_c, phi_c, amp_c = gamma[sel], phi[sel], amp[sel]

        # phase constants per (atom, 960-chunk); y in cycles incl. +0.25
        D = t_cen[None, :] - tau_c[:, None]                  # [256, 100]
        y = om_c[:, None] * D + ga_c[:, None] * D * D / (2 * np.pi) \
            + (phi_c[:, None] / (2 * np.pi)) + 0.25
        dy = om_c[:, None] + ga_c[:, None] * D / np.pi       # cycles/sec
        bi = np.round(dy / FS * FIX)                         # fix units/sample
        ci = np.round(np.mod(y, 1.0) * FIX)                  # [0, 2^23]

        # envelope (with amp and the -sin sign fold) at chunk nodes
        E = amp_c[:, None] * np.exp(
            -0.5 * ((t_node[None, :] - tau_c[:, None]) / sig_c[:, None]) ** 2)
        e0 = -E[:, :-1]                                      # [256, 375]
        ed = -(E[:, 1:] - E[:, :-1])
        envp = np.empty((per_core, 2 * N_COLS), np.float64)
        envp[:, 0::2] = e0
        envp[:, 1::2] = ed

        def blk(x):  # [256, W] -> [128, 2*W] block-major
            w = x.shape[1]
            out = np.empty((P, BLOCKS * w), x.dtype)
            for b in range(BLOCKS):
                out[:, b * w:(b + 1) * w] = x[b * P:(b + 1) * P]
            return out

        in_maps.append({
            "bi": np.ascontiguousarray(blk(bi).astype(np.float32)),
            "ci": np.ascontiguousarray(blk(ci).astype(np.float32)),
            "envp": np.ascontiguousarray(blk(envp).astype(bft)),
        })
    return in_maps


def kernel(amplitude_logit, tau, omega_logit, sigma_logit, phi_vector, gamma, t):
    if "nc" not in _cache:
        _cache["nc"] = _build_program()
    nc = _cache["nc"]
    in_maps = _prepare_inputs(amplitude_logit, tau, omega_logit, sigma_logit,
                              phi_vector, gamma)
    res = run_bass_kernel_spmd(nc, in_maps, list(range(N_CORES)))
    total = np.zeros(T, dtype=np.float64)
    for r in res.results:
        wv = r["wave"].astype(np.float64)          # [P, N_COLS]
        total += wv.T.ravel()                      # sample s = c*128 + i
    return total.astype(np.float32)


# revision 5
# speedup vs baseline: 3.4363x; 1.2951x over previous
"""Gabor-atom additive audio synthesis on 8 Trainium2 NeuronCores.

Math: waveform[t] = sum_n amp_n * exp(-0.5*((t-tau_n)/sigma_n)^2)
                    * cos(2*pi*omega_n*(t-tau_n) + gamma_n*(t-tau_n)^2 + phi_n)
with N=2048 atoms, T=48000 samples (2s @ 24kHz).

Sharding: atoms across 8 cores (256 atoms/core = 2 blocks of 128 partitions).

Per-core pipeline (DDS-style fixed-point phase, 1 cycle = 2^23):
  - DVE/GPSIMD tensor_scalar FMA: k = round_i32(Bi[p]*i' + Ci[p]) per
    960-sample chunk (centered ramp i' in [-480,480) keeps |v| < 2^31; Bi/Ci
    are integer-valued f32 so the f32 FMA is exact to ~2^-24 of the phase)
  - DVE fused (k & 0x7FFFFF) | 0x3F800000: IEEE bits of 1 + frac(phase);
    bitcast to f32 gives the fractional phase linearly in [1,2)
  - ACT Sin(2*pi*x - 3*pi) on the bitcast: -sin(2*pi*frac) = -cos(orig phase)
    (the +0.25-cycle cos->sin shift is folded into Ci on the host)
  - PE reduce per 128-sample chunk: out[i,{0,1}] = sum_p sin[p,i]*envp[p,{0,1}]
    with envp = (-amp*env(node c), -amp*(env(node c+1)-env(node c))) host-
    computed at chunk boundaries (the Gaussian env is smooth at 5.3ms scale)
  - DVE combine: wave[i,c] = r0 + (i/128)*rd  (linear env interpolation)
Schedule: Sin table preloaded by a 1-element warm-up; tile 0 runs split in
two halves (896/1024, all-DVE FMAs) so ACT starts ~4us in; tiles 1..24 batch
Sin per 2-tile pair; the combine+output DMA runs in two halves so only half
remains after the last Sin. Host: fp64 transforms + per-chunk phase/envelope
constants, final 8-way sum.
"""
import numpy as np
import ml_dtypes
from contextlib import ExitStack

import concourse.bacc as bacc
import concourse.tile as tile
from concourse import mybir
from concourse.bass_utils import run_bass_kernel_spmd

# ---- problem constants (hardcoded; kernel.py must be self-contained) ----
FS = 24000.0
T = 48000
N_ATOMS = 2048
N_CORES = 8
NYQUIST = FS / 2.0
SIGMA_OFFSET = 1e-3

P = 128                      # partitions / atoms per block
BLOCKS = 2                   # atom blocks per core (256 atoms/core)
FIX = 2 ** 23                # fixed-point: 1 cycle = 2^23
CH = 960                     # FMA chunk (|Bi*i'| <= .495*2^23*480 < 2^31)
N_CH = T // CH               # 100 FMA chunks
F = 1920                     # processing tile = 2 FMA chunks
N_TILES = T // F             # 25
RCH = 128                    # reduce chunk (output column height)
N_COLS = T // RCH            # 375
RPT = F // RCH               # reduce chunks per tile (15)
HA = 896                     # tile-0 half A samples (7 reduce chunks)
HB = F - HA                  # tile-0 half B samples (1024, 8 chunks)

f32 = mybir.dt.float32
i32 = mybir.dt.int32
bf16 = mybir.dt.bfloat16
bft = ml_dtypes.bfloat16

_cache = {}


def _build_program():
    nc = bacc.Bacc("TRN2", target_bir_lowering=False, debug=False)

    d_bi = nc.dram_tensor("bi", [P, BLOCKS * N_CH], f32, kind="ExternalInput").ap()
    d_ci = nc.dram_tensor("ci", [P, BLOCKS * N_CH], f32, kind="ExternalInput").ap()
    d_envp = nc.dram_tensor("envp", [P, BLOCKS * 2 * N_COLS], bf16,
                            kind="ExternalInput").ap()
    d_out = nc.dram_tensor("wave", [P, N_COLS], f32, kind="ExternalOutput").ap()

    with tile.TileContext(nc) as tc, ExitStack() as ctx:
        consts = ctx.enter_context(tc.tile_pool(name="consts", bufs=1))
        kpool = ctx.enter_context(tc.tile_pool(name="kp", bufs=3))
        mpool = ctx.enter_context(tc.tile_pool(name="mp", bufs=2))
        spool = ctx.enter_context(tc.tile_pool(name="sp", bufs=2))
        opool = ctx.enter_context(tc.tile_pool(name="op", bufs=1))
        rpool = ctx.enter_context(tc.tile_pool(name="rp", bufs=1, space="PSUM"))

        # centered local ramp i' in [-CH/2, CH/2) first: it gates the FMAs
        # (used directly as i32 FMA input; the ALU upcasts exactly)
        t_ji = consts.tile([P, CH], i32)
        nc.gpsimd.iota(t_ji[:], [[1, CH]], base=-CH // 2, channel_multiplier=0)
        t_pi = consts.tile([P, 1], i32)
        nc.gpsimd.iota(t_pi[:], [[0, 1]], base=0, channel_multiplier=1)

        t_bias = consts.tile([P, 1], f32)
        nc.vector.memset(t_bias[:], float(-3.0 * np.pi))
        # preload the Sin act table while the pipeline fills
        t_warm = consts.tile([P, 1], bf16)
        nc.scalar.activation(t_warm[:], t_bias[:],
                             mybir.ActivationFunctionType.Sin, scale=1.0)

        t_bi = consts.tile([P, BLOCKS * N_CH], f32)
        nc.sync.dma_start(t_bi[:], d_bi[:])
        t_ci = consts.tile([P, BLOCKS * N_CH], f32)
        nc.sync.dma_start(t_ci[:], d_ci[:])
        t_envp = consts.tile([P, BLOCKS * 2 * N_COLS], bf16)
        half = N_COLS  # half the pairs of each block
        nc.sync.dma_start(t_envp[:, :half], d_envp[:, :half])
        nc.sync.dma_start(t_envp[:, BLOCKS * N_COLS:BLOCKS * N_COLS + half],
                          d_envp[:, BLOCKS * N_COLS:BLOCKS * N_COLS + half])
        nc.sync.dma_start(t_envp[:, half:BLOCKS * N_COLS],
                          d_envp[:, half:BLOCKS * N_COLS])
        nc.sync.dma_start(t_envp[:, BLOCKS * N_COLS + half:],
                          d_envp[:, BLOCKS * N_COLS + half:])

        # per-partition ramp weight i/128 for the env interpolation
        t_w = consts.tile([P, 1], f32)
        nc.vector.tensor_scalar(t_w[:], t_pi[:], float(1.0 / RCH), None,
                                mybir.AluOpType.mult)

        p_r = rpool.tile([P, 2 * N_COLS], f32)
        t_wav = opool.tile([P, N_COLS], f32)

        # GPSIMD takes 54 of the 96 FMA items of tiles 1..24 (9/16);
        # tile 0's 4 items stay on DVE for the fastest possible head.
        def pick_engine(q):
            return nc.gpsimd if (q * 9) % 16 < 9 else nc.vector

        def fma_item(t_k, t, b, h, eng):
            col = b * N_CH + t * 2 + h
            eng.tensor_scalar(
                t_k[:, b * F + h * CH: b * F + (h + 1) * CH],
                t_ji[:], t_bi[:, col:col + 1], t_ci[:, col:col + 1],
                mybir.AluOpType.mult, mybir.AluOpType.add)

        def andor(t_m, moff, t_k, koff, w):
            nc.vector.tensor_scalar(
                t_m[:, moff:moff + w], t_k[:, koff:koff + w],
                0x7FFFFF, 0x3F800000,
                mybir.AluOpType.bitwise_and, mybir.AluOpType.bitwise_or)

        def sin(t_s, t_m, off, w):
            nc.scalar.activation(
                t_s[:, off:off + w], t_m[:, off:off + w].bitcast(f32),
                mybir.ActivationFunctionType.Sin,
                scale=float(2.0 * np.pi), bias=t_bias[:, 0:1])

        def rmm(c, stat_ap, b):
            nc.tensor.matmul(
                p_r[:, 2 * c:2 * c + 2], stat_ap,
                t_envp[:, b * 2 * N_COLS + 2 * c: b * 2 * N_COLS + 2 * c + 2],
                start=(b == 0), stop=(b == BLOCKS - 1))

        def combine(lo, hi):  # wave[i,c] = r0[i,c] + (i/128)*rd[i,c]
            t_tmp = opool.tile([P, N_COLS], f32, tag="tmp")
            nc.vector.tensor_scalar(
                t_tmp[:, lo:hi], p_r[:, 2 * lo + 1:2 * hi:2], t_w[:, 0:1],
                None, mybir.AluOpType.mult)
            nc.vector.tensor_tensor(t_wav[:, lo:hi], t_tmp[:, lo:hi],
                                    p_r[:, 2 * lo:2 * hi:2],
                                    mybir.AluOpType.add)
            nc.sync.dma_start(d_out[:, lo:hi], t_wav[:, lo:hi])

        # --- tile 0: split halves (A: samples 0..895, B: 896..1919),
        # all-DVE FMAs; GPSIMD starts on tiles 1/2 concurrently ---
        t_k0 = kpool.tile([P, BLOCKS * F], i32, tag="k")
        t_m0 = mpool.tile([P, 2 * BLOCKS * F], i32, tag="m")
        t_s0 = spool.tile([P, 2 * BLOCKS * F], bf16, tag="s")
        for b in range(BLOCKS):          # half A: needs only h=0 FMAs
            fma_item(t_k0, 0, b, 0, nc.vector)
        for b in range(BLOCKS):
            andor(t_m0, b * HA, t_k0, b * F, HA)
        sin(t_s0, t_m0, 0, BLOCKS * HA)
        for b in range(BLOCKS):
            fma_item(t_k0, 0, b, 1, nc.vector)
        for b in range(BLOCKS):          # half B
            andor(t_m0, BLOCKS * HA + b * HB, t_k0, b * F + HA, HB)
        sin(t_s0, t_m0, BLOCKS * HA, BLOCKS * HB)
        for r in range(RPT):
            for b in range(BLOCKS):
                if r < HA // RCH:
                    off = b * HA + r * RCH
                else:
                    off = BLOCKS * HA + b * HB + (r - HA // RCH) * RCH
                rmm(r, t_s0[:, off:off + RCH], b)

        # --- tiles 1 and 2: single-tile Sins to keep ACT fed early ---
        q = 0
        for t in (1, 2):
            t_k = kpool.tile([P, BLOCKS * F], i32, tag="k")
            for b in range(BLOCKS):
                for h in range(2):
                    fma_item(t_k, t, b, h, pick_engine(q))
                    q += 1
            t_m = mpool.tile([P, 2 * BLOCKS * F], i32, tag="m")
            t_s = spool.tile([P, 2 * BLOCKS * F], bf16, tag="s")
            andor(t_m, 0, t_k, 0, BLOCKS * F)
            sin(t_s, t_m, 0, BLOCKS * F)
            for r in range(RPT):
                c = t * RPT + r
                for b in range(BLOCKS):
                    rmm(c, t_s[:, b * F + r * RCH: b * F + (r + 1) * RCH], b)

        # --- tiles 3..24: Sin batched per pair ---
        t = 3
        while t < N_TILES:
            pair = min(2, N_TILES - t)
            t_m = mpool.tile([P, 2 * BLOCKS * F], i32, tag="m")
            t_s = spool.tile([P, 2 * BLOCKS * F], bf16, tag="s")
            for pi in range(pair):
                t_k = kpool.tile([P, BLOCKS * F], i32, tag="k")
                for b in range(BLOCKS):
                    for h in range(2):
                        fma_item(t_k, t + pi, b, h, pick_engine(q))
                        q += 1
                andor(t_m, pi * BLOCKS * F, t_k, 0, BLOCKS * F)
            sin(t_s, t_m, 0, pair * BLOCKS * F)
            for pi in range(pair):
                for r in range(RPT):
                    c = (t + pi) * RPT + r
                    b_off = pi * BLOCKS * F
                    for b in range(BLOCKS):
                        rmm(c, t_s[:, b_off + b * F + r * RCH:
                                   b_off + b * F + (r + 1) * RCH], b)
            t += pair
            if t == 13:                  # chunks 0..194 are final
                combine(0, 13 * RPT)
            elif t == 23:                # chunks 195..344 are final
                combine(13 * RPT, 23 * RPT)

        combine(23 * RPT, N_COLS)

    nc.compile()
    return nc


def _prepare_inputs(amplitude_logit, tau, omega_logit, sigma_logit,
                    phi_vector, gamma):
    """fp64 host prep -> per-core input maps."""
    al = amplitude_logit.astype(np.float64)
    tau = tau.astype(np.float64)
    ol = omega_logit.astype(np.float64)
    sl = sigma_logit.astype(np.float64)
    pv = phi_vector.astype(np.float64)
    gamma = gamma.astype(np.float64)

    amp = np.where(al > 30, al, np.log1p(np.exp(al)))
    omega = (1.0 / (1.0 + np.exp(-ol))) * 0.99 * NYQUIST
    sigma = np.where(sl > 30, sl, np.log1p(np.exp(sl))) + SIGMA_OFFSET
    phi = np.arctan2(pv[:, 1], pv[:, 0])

    t_cen = (np.arange(N_CH) * CH + CH // 2) / FS            # [100]
    t_node = np.arange(N_COLS + 1) * RCH / FS                # [376]

    per_core = N_ATOMS // N_CORES
    in_maps = []
    for c in range(N_CORES):
        sel = slice(c * per_core, (c + 1) * per_core)
        tau_c, sig_c, om_c = tau[sel], sigma[sel], omega[sel]
        ga_c, phi_c, amp_c = gamma[sel], phi[sel], amp[sel]

        # phase constants per (atom, 960-chunk); y in cycles incl. +0.25
        D = t_cen[None, :] - tau_c[:, None]                  # [256, 100]
        y = om_c[:, None] * D + ga_c[:, None] * D * D / (2 * np.pi) \
            + (phi_c[:, None] / (2 * np.pi)) + 0.25
        dy = om_c[:, None] + ga_c[:, None] * D / np.pi       # cycles/sec
        bi = np.round(dy / FS * FIX)                         # fix units/sample
        ci = np.round(np.mod(y, 1.0) * FIX)                  # [0, 2^23]

        # envelope (with amp and the -sin sign fold) at chunk nodes
        E = amp_c[:, None] * np.exp(
            -0.5 * ((t_node[None, :] - tau_c[:, None]) / sig_c[:, None]) ** 2)
        e0 = -E[:, :-1]                                      # [256, 375]
        ed = -(E[:, 1:] - E[:, :-1])
        envp = np.empty((per_core, 2 * N_COLS), np.float64)
        envp[:, 0::2] = e0
        envp[:, 1::2] = ed

        def blk(x):  # [256, W] -> [128, 2*W] block-major
            w = x.shape[1]
            out = np.empty((P, BLOCKS * w), x.dtype)
            for b in range(BLOCKS):
                out[:, b * w:(b + 1) * w] = x[b * P:(b + 1) * P]
            return out

        in_maps.append({
            "bi": np.ascontiguousarray(blk(bi).astype(np.float32)),
            "ci": np.ascontiguousarray(blk(ci).astype(np.float32)),
            "envp": np.ascontiguousarray(blk(envp).astype(bft)),
        })
    return in_maps


def kernel(amplitude_logit, tau, omega_logit, sigma_logit, phi_vector, gamma, t):
    if "nc" not in _cache:
        _cache["nc"] = _build_program()
    nc = _cache["nc"]
    in_maps = _prepare_inputs(amplitude_logit, tau, omega_logit, sigma_logit,
                              phi_vector, gamma)
    res = run_bass_kernel_spmd(nc, in_maps, list(range(N_CORES)))
    total = np.zeros(T, dtype=np.float64)
    for r in res.results:
        wv = r["wave"].astype(np.float64)          # [P, N_COLS]
        total += wv.T.ravel()                      # sample s = c*128 + i
    return total.astype(np.float32)
